# revision 5
# baseline (speedup 1.0000x reference)
"""Bass/Trainium2 kernel for nn_EnhancedMultiHeadAttention (sparse_attention).

out[b,h,i,j] = softmax_j( (q_bh i . k_bh j) * sc + relbias[b,i,j] + mask_term[b,i,j] )
  q = query @ Wq.T + bq   (sc = 1/sqrt(64) folded into Wq/bq on host)
  relbias[b,i,j] = (mean_h q[b,h,i,:]) . rel_k_table[clip(j-i,-128,128)+128, :] * sc
  mask_term = 0 where mask==1 else -1e9

Sharding: 8 cores = 4 batches x 2 head-halves (8 heads per core).
Host pre-transposes per-batch activations (query[b].T) so the contraction dim
(D) lands on SBUF partitions; the head-mean projection is folded into a
[64,1024] weight since the projection is linear.

Per-core device program (all matmuls in float32r = full-rate fp32):
  1. qT[512,S], kT[512,S], qmT[64,S] projections (PSUM accumulate over D,
     bias added during ACT evacuation).
  2. W = qm @ rel_k_table.T -> [S,257]; pad edges (clip regions) to [S,511];
     bounce through DRAM and read back with a skewed access pattern
     (partition step 510) to materialize the diagonal band bias[i, j-i+128].
  3. combined[b] = (mask-1)*1e9 + relbias, assembled once per batch
     ([S,S] in SBUF), shared by all 8 heads.
  4. Per (head, row-tile): scores PSUM = qk matmuls (K=64, two heads packed
     per PE pass) + identity-matmul accumulate of combined; ACT exp with
     accum_out row sums; DVE reciprocal + per-partition normalize; DMA out.
"""

import numpy as np

B, S, D, H = 4, 1024, 1024, 16
DK = 64          # head dim
MAXREL = 128
NREL = 2 * MAXREL + 1          # 257
WPADW = 2 * MAXREL + NREL - 2  # 511 = 127 + 257 + 127
NRELP = 260     # rel matmul free dim padded for fp32r ISA restrictions
HPC = 8          # heads per core
DHALF = 512      # projected dims per core
NCORES = 8
PT = 128         # partition tile
NT = S // PT     # 8 row tiles

_CACHE = {}


def _build():
    from contextlib import ExitStack

    import concourse.bass as bass
    import concourse.mybir as mybir
    import concourse.tile as tile
    from concourse import bacc
    from concourse.tile import add_dep_helper

    F32 = mybir.dt.float32
    F32R = mybir.dt.float32r
    I32 = mybir.dt.int32
    AF = mybir.ActivationFunctionType

    nc = bacc.Bacc("TRN2", target_bir_lowering=False, debug=False)

    xT = nc.dram_tensor("xT", [D, S], F32, kind="ExternalInput")
    kTx = nc.dram_tensor("kTx", [D, S], F32, kind="ExternalInput")
    maskb = nc.dram_tensor("maskb", [S, S], I32, kind="ExternalInput")
    wqT = nc.dram_tensor("wqT", [D, DHALF], F32, kind="ExternalInput")
    wkT = nc.dram_tensor("wkT", [D, DHALF], F32, kind="ExternalInput")
    bq4 = nc.dram_tensor("bq4", [PT, 4], F32, kind="ExternalInput")
    bk4 = nc.dram_tensor("bk4", [PT, 4], F32, kind="ExternalInput")
    wmT = nc.dram_tensor("wmT", [D, DK], F32, kind="ExternalInput")
    bm1 = nc.dram_tensor("bm1", [DK, 1], F32, kind="ExternalInput")
    tT = nc.dram_tensor("tT", [DK, NRELP], F32, kind="ExternalInput")
    out_d = nc.dram_tensor("out", [HPC, S, S], F32, kind="ExternalOutput")
    wpad_d = nc.dram_tensor("wpad_scratch", [S, WPADW], F32)
    ident_d = nc.inline_tensor(np.eye(PT, dtype=np.float32), "ident")

    with tile.TileContext(nc) as tc, ExitStack() as ctx:
        wpool = ctx.enter_context(tc.tile_pool(name="wpool", bufs=8))
        wmpool = ctx.enter_context(tc.tile_pool(name="wmpool", bufs=8))
        xpool = ctx.enter_context(tc.tile_pool(name="xpool", bufs=8))
        persist = ctx.enter_context(tc.tile_pool(name="persist", bufs=1))
        mpool = ctx.enter_context(tc.tile_pool(name="mpool", bufs=2))
        bpool = ctx.enter_context(tc.tile_pool(name="bpool", bufs=1))
        epool = ctx.enter_context(tc.tile_pool(name="epool", bufs=3))
        opool = ctx.enter_context(tc.tile_pool(name="opool", bufs=3))
        spool = ctx.enter_context(tc.tile_pool(name="spool", bufs=4))
        psum = ctx.enter_context(tc.tile_pool(name="psum", bufs=2, space="PSUM"))

        # ---- small constants ----
        id_sb = persist.tile([PT, PT], F32R, tag="ident")
        nc.sync.dma_start(id_sb[:], ident_d[:].bitcast(F32R))
        bq_sb = persist.tile([PT, 4], F32, tag="bq")
        nc.sync.dma_start(bq_sb[:], bq4[:])
        bk_sb = persist.tile([PT, 4], F32, tag="bk")
        nc.sync.dma_start(bk_sb[:], bk4[:])
        bm_sb = persist.tile([DK, 1], F32, tag="bm")
        nc.sync.dma_start(bm_sb[:], bm1[:])
        tT_sb = persist.tile([DK, NRELP], F32R, tag="tT")
        nc.sync.dma_start(tT_sb[:], tT[:].bitcast(F32R))
        ones_sb = persist.tile([PT, MAXREL - 1], F32, tag="ones")
        nc.vector.memset(ones_sb[:], 1.0)

        # ---- mean-projection weights ----
        wm_sb = []
        for kc in range(NT):
            t = wmpool.tile([PT, DK], F32R, tag="wm")
            nc.sync.dma_start(t[:], wmT[kc * PT:(kc + 1) * PT, :].bitcast(F32R))
            wm_sb.append(t)

        # ---- q/k projection weights (k reuses q's slots via shared tag) ----
        def load_w(src):
            tiles = []
            for kc in range(NT):
                t = wpool.tile([PT, DHALF], F32R, tag="w")
                nc.sync.dma_start(t[:], src[kc * PT:(kc + 1) * PT, :].bitcast(F32R))
                tiles.append(t)
            return tiles

        def load_x(src):
            tiles = []
            for kc in range(NT):
                t = xpool.tile([PT, S], F32R, tag="x")
                nc.sync.dma_start(t[:], src[kc * PT:(kc + 1) * PT, :].bitcast(F32R))
                tiles.append(t)
            return tiles

        # ---- projections: proj[db][:, nh] = W[db-block] @ x.T + b ----
        def project(w_tiles, x_tiles, dst_tiles, bias_sb):
            for nh in range(2):
                for db in range(4):
                    ps = psum.tile([PT, DHALF], F32, tag="psA")
                    for kc in range(NT):
                        nc.tensor.matmul(
                            ps[:],
                            w_tiles[kc][:, db * PT:(db + 1) * PT],
                            x_tiles[kc][:, nh * DHALF:(nh + 1) * DHALF],
                            start=(kc == 0), stop=(kc == NT - 1),
                        )
                    nc.scalar.activation(
                        dst_tiles[db][:, nh * DHALF:(nh + 1) * DHALF], ps[:],
                        AF.Identity, bias=bias_sb[:, db:db + 1], scale=1.0)

        qT_sb = [persist.tile([PT, S], F32R, tag=f"qT{i}", name=f"qT{i}") for i in range(4)]
        kT_sb = [persist.tile([PT, S], F32R, tag=f"kT{i}", name=f"kT{i}") for i in range(4)]
        qmT_sb = persist.tile([DK, S], F32R, tag="qmT")

        wq_tiles = load_w(wqT)
        x_tiles = load_x(xT)
        project(wq_tiles, x_tiles, qT_sb, bq_sb)

        # head-mean projection qmT[64, S]
        for nh in range(2):
            ps = psum.tile([DK, DHALF], F32, tag="psB")
            for kc in range(NT):
                nc.tensor.matmul(
                    ps[:], wm_sb[kc][:], x_tiles[kc][:, nh * DHALF:(nh + 1) * DHALF],
                    start=(kc == 0), stop=(kc == NT - 1))
            nc.scalar.activation(
                qmT_sb[:, nh * DHALF:(nh + 1) * DHALF], ps[:],
                AF.Identity, bias=bm_sb[:], scale=1.0)

        wk_tiles = load_w(wkT)
        k_tiles = load_x(kTx)
        project(wk_tiles, k_tiles, kT_sb, bk_sb)

        # ---- rel-position bias: W = qm @ T.T, pad, skew via DRAM ----
        band_info = []  # (jlo, jhi) per row tile
        for m in range(NT):
            jlo = max(0, PT * (m - 1))
            jhi = min(S, PT * (m + 2))
            band_info.append((jlo, jhi))

        wpad_sb = [persist.tile([PT, WPADW], F32, tag=f"wpad{m}", name=f"wpad{m}") for m in range(NT)]
        w0_sb = [persist.tile([PT, 1], F32, tag=f"w0_{m}", name=f"w0_{m}") for m in range(NT)]
        w256_sb = [persist.tile([PT, 1], F32, tag=f"w256_{m}", name=f"w256_{m}") for m in range(NT)]
        band_sb = []
        for m in range(NT):
            ps = psum.tile([PT, NRELP], F32, tag="psB")
            nc.tensor.matmul(ps[:], qmT_sb[:, m * PT:(m + 1) * PT], tT_sb[:],
                             start=True, stop=True)
            nc.scalar.copy(wpad_sb[m][:, MAXREL - 1:MAXREL - 1 + NREL], ps[:, 0:NREL])
            nc.vector.tensor_copy(w0_sb[m][:], ps[:, 0:1])
            nc.vector.tensor_copy(w256_sb[m][:], ps[:, NREL - 1:NREL])
            nc.vector.tensor_scalar_mul(
                wpad_sb[m][:, 0:MAXREL - 1], ones_sb[:], w0_sb[m][:])
            nc.vector.tensor_scalar_mul(
                wpad_sb[m][:, MAXREL - 1 + NREL:WPADW], ones_sb[:], w256_sb[m][:])
            wi = nc.sync.dma_start(wpad_d[m * PT:(m + 1) * PT, :], wpad_sb[m][:])
            # skewed read back: band[p, jj] = wpad[m*128+p, (jlo+jj) - (m*128+p) + 255]
            jlo, jhi = band_info[m]
            bt = bpool.tile([PT, jhi - jlo], F32, tag=f"band{m}", name=f"band{m}")
            src = bass.AP(wpad_d, PT * (WPADW - 1) * m + jlo + (WPADW // 2),
                          [[WPADW - 1, PT], [1, jhi - jlo]])
            ri = nc.sync.dma_start(bt[:], src)
            add_dep_helper(ri.ins, wi.ins, reason="wpad DRAM RAW")
            band_sb.append(bt)

        # ---- combined bias per row tile: (mask-1)*1e9 + relbias ----
        comb_sb = []
        for m in range(NT):
            jlo, jhi = band_info[m]
            mt = mpool.tile([PT, S], I32, tag="mask")
            nc.sync.dma_start(mt[:], maskb[m * PT:(m + 1) * PT, :])
            cb = persist.tile([PT, S], F32R, tag=f"comb{m}")
            nc.scalar.activation(cb[:], mt[:], AF.Copy, bias=-1e9, scale=1e9)
            nc.vector.tensor_add(cb[:, jlo:jhi], cb[:, jlo:jhi], band_sb[m][:])
            if jlo > 0:
                nc.vector.tensor_scalar_add(cb[:, 0:jlo], cb[:, 0:jlo], w0_sb[m][:])
            if jhi < S:
                nc.vector.tensor_scalar_add(cb[:, jhi:S], cb[:, jhi:S], w256_sb[m][:])
            comb_sb.append(cb)

        # ---- main loop: scores + softmax per (head pair, row tile) ----
        for t in range(4):
            for m in range(NT):
                mb = slice(m * PT, (m + 1) * PT)
                ps0 = psum.tile([PT, S], F32, tag="psA")
                ps1 = psum.tile([PT, S], F32, tag="psB")
                for nh in range(2):
                    nhs = slice(nh * DHALF, (nh + 1) * DHALF)
                    nc.tensor.matmul(ps0[:, nhs], qT_sb[t][0:DK, mb],
                                     kT_sb[t][0:DK, nhs], start=True, stop=False)
                    nc.tensor.matmul(ps1[:, nhs], qT_sb[t][DK:PT, mb],
                                     kT_sb[t][DK:PT, nhs], start=True, stop=False)
                for nh in range(2):
                    nhs = slice(nh * DHALF, (nh + 1) * DHALF)
                    nc.tensor.matmul(ps0[:, nhs], id_sb[:], comb_sb[m][:, nhs],
                                     start=False, stop=True)
                    nc.tensor.matmul(ps1[:, nhs], id_sb[:], comb_sb[m][:, nhs],
                                     start=False, stop=True)
                for hi, ps in ((0, ps0), (1, ps1)):
                    h = 2 * t + hi
                    e = epool.tile([PT, S], F32, tag=f"e{hi}")
                    sm = spool.tile([PT, 1], F32, tag=f"s{hi}")
                    nc.scalar.activation(e[:], ps[:], AF.Exp, bias=0.0, scale=1.0,
                                         accum_out=sm[:])
                    r = spool.tile([PT, 1], F32, tag=f"r{hi}")
                    nc.vector.reciprocal(r[:], sm[:])
                    o = opool.tile([PT, S], F32, tag=f"o{hi}")
                    nc.vector.tensor_scalar_mul(o[:], e[:], r[:])
                    nc.sync.dma_start(out_d[h, mb, :], o[:])

    nc.compile()
    return nc


def _get_nc():
    if "nc" not in _CACHE:
        _CACHE["nc"] = _build()
    return _CACHE["nc"]


def _prep_inputs(query, key, mask, Wq, bq, Wk, bk, rel_k_table):
    """Host-side sharding prep -> 8 per-core input dicts."""
    sc = 1.0 / np.sqrt(np.float32(DK))
    query = np.asarray(query, dtype=np.float32)
    key = np.asarray(key, dtype=np.float32)
    mask = np.ascontiguousarray(np.asarray(mask, dtype=np.int32))
    Wq = np.asarray(Wq, dtype=np.float32)
    bq = np.asarray(bq, dtype=np.float32)
    Wk = np.asarray(Wk, dtype=np.float32)
    bk = np.asarray(bk, dtype=np.float32)
    T = np.asarray(rel_k_table, dtype=np.float32)

    WqTs = np.ascontiguousarray((Wq * sc).T)       # [D, D]
    WkT = np.ascontiguousarray(Wk.T)               # [D, D]
    bqs = bq * sc
    Wm = np.ascontiguousarray((Wq.reshape(H, DK, D).mean(0) * sc).T)  # [D, 64]
    bm = (bq.reshape(H, DK).mean(0) * sc).reshape(DK, 1).astype(np.float32)
    tTc = np.zeros((DK, NRELP), np.float32)
    tTc[:, :NREL] = T.T                            # [64, 260] zero-padded

    xT = [np.ascontiguousarray(query[b].T) for b in range(B)]
    kT = [np.ascontiguousarray(key[b].T) for b in range(B)]

    in_maps = []
    for c in range(NCORES):
        b, hh = divmod(c, 2)
        cols = slice(hh * DHALF, (hh + 1) * DHALF)
        in_maps.append(dict(
            xT=xT[b], kTx=kT[b], maskb=mask[b],
            wqT=np.ascontiguousarray(WqTs[:, cols]),
            wkT=np.ascontiguousarray(WkT[:, cols]),
            bq4=np.ascontiguousarray(bqs[cols].reshape(4, PT).T),
            bk4=np.ascontiguousarray(bk[cols].reshape(4, PT).T),
            wmT=Wm, bm1=bm, tT=tTc,
        ))
    return in_maps


def run(inputs: dict, trace: bool = False):
    from concourse.bass_utils import run_bass_kernel_spmd

    nc = _get_nc()
    in_maps = _prep_inputs(**inputs)
    res = run_bass_kernel_spmd(nc, in_maps, core_ids=list(range(NCORES)),
                               trace=trace)
    out = np.empty((B, H, S, S), dtype=np.float32)
    for c in range(NCORES):
        b, hh = divmod(c, 2)
        out[b, hh * HPC:(hh + 1) * HPC] = res.results[c]["out"]
    return out, res


def kernel(**inputs) -> np.ndarray:
    out, _ = run(inputs)
    return out
